# revision 15
# baseline (speedup 1.0000x reference)
"""2D Gaussian splat rasterizer on 8 Trainium2 NeuronCores — v2.

Strategy: shard the image into tiles of SR x TC pixels (F = SR*TC pixels
per tile), n_slots tiles per core. Gaussians are culled host-side per
tile by their raster_ratio-sigma bounding box; per-tile counts (raw, no
quantization) form one canonical slot profile shared by all 8 cores
(SPMD). Slots are grouped into "octets" of OCT slots whose gaussians are
concatenated and padded to a multiple of 128 (the chunk size), so no
chunk crosses an octet boundary. Per chunk of 128 gaussians:

    arg   = coefT.T @ basis        TensorE, K=12 fp16 hi/lo split of the
                                   6-term pixel basis [x^2, xy, y^2, x, y, 1]
                                   (tile-local coords; ln(opacity) folded in)
    alpha = Exp(arg)               ScalarE, fused over a GROUP of chunks
                                   spanning 3 PSUM banks, fp16 out
    acc  += colorsT @ alpha        TensorE, ONE K=128 M=3*OCT matmul per
                                   chunk: each slot owns a fixed 3-row
                                   triplet in the octet's accumulator
                                   region; gaussians outside a slot get
                                   zero color columns, so arbitrary slot
                                   boundaries are fine (the quadratic form
                                   is <= 0 everywhere, so stray alphas are
                                   bounded by 1 and killed by zero colors)

Accumulator regions (24 rows x F cols) live in TWO PSUM banks with
octets alternating A/B in chunk order, so each drain (VectorE copy +
DMA, issued as soon as the octet's last chunk closes) overlaps PE writes
to the OTHER bank only — PE-write + DVE-read of one PSUM bank is fatal
on hardware. All color matmuls share one tile_position and full-K
partition ranges (identical-range matmuls at different tile positions
get co-issued into a conflicting PE group and hang the device), and arg
matmuls are emitted bank-interleaved so the row-group-concurrent
neighbors never write the same PSUM bank. Inputs arrive on three queues
(SP / Activation HWDGE + GPSIMD SWDGE) so both coef row groups land in
parallel; a dummy Exp pulls the ACT table load forward under the input
DMAs. Output is [24, n_oct*F] fp16 per core — only meaningful rows. The
full [H, W, 3] image is reassembled host-side (pure concatenation).
"""

import numpy as np
import concourse.bacc as bacc
import concourse.tile as tile
from concourse import mybir
from concourse.bass_utils import run_bass_kernel_spmd

_runner_cache = {}


def _get_runner(nc):
    """Persistent jitted SPMD executor for a compiled Bass program."""
    key = id(nc)
    if key in _runner_cache:
        return _runner_cache[key]
    import jax
    from jax.sharding import Mesh, PartitionSpec
    from jax.experimental.shard_map import shard_map
    from concourse import bass2jax, mybir as mb

    bass2jax.install_neuronx_cc_hook()

    in_names, out_names, out_avals, zero_outs = [], [], [], []
    partition_name = nc.partition_id_tensor.name if nc.partition_id_tensor else None
    for alloc in nc.m.functions[0].allocations:
        if not isinstance(alloc, mb.MemoryLocationSet):
            continue
        name = alloc.memorylocations[0].name
        if alloc.kind == "ExternalInput":
            if name != partition_name:
                in_names.append(name)
        elif alloc.kind == "ExternalOutput":
            shape = tuple(alloc.tensor_shape)
            dtype = mb.dt.np(alloc.dtype)
            out_names.append(name)
            out_avals.append(jax.core.ShapedArray(shape, dtype))
            zero_outs.append(np.zeros(shape, dtype))
    n_params = len(in_names)
    all_in = in_names + out_names + ([partition_name] if partition_name else [])

    def _body(*args):
        operands = list(args)
        if partition_name is not None:
            operands.append(bass2jax.partition_id_tensor())
        outs = bass2jax._bass_exec_p.bind(
            *operands,
            out_avals=tuple(out_avals),
            in_names=tuple(all_in),
            out_names=tuple(out_names),
            lowering_input_output_aliases=(),
            sim_require_finite=True,
            sim_require_nnan=True,
            nc=nc,
        )
        return tuple(outs)

    devices = jax.devices()[:N_CORES]
    mesh = Mesh(np.asarray(devices), ("core",))
    in_specs = (PartitionSpec("core"),) * (n_params + len(out_names))
    out_specs = (PartitionSpec("core"),) * len(out_names)
    sharded = jax.jit(
        shard_map(
            _body, mesh=mesh, in_specs=in_specs, out_specs=out_specs, check_rep=False
        ),
        donate_argnums=tuple(range(n_params, n_params + len(out_names))),
        keep_unused=True,
    )

    def run(in_maps):
        concat_in = [
            np.concatenate([np.asarray(m[nm]) for m in in_maps], axis=0)
            for nm in in_names
        ]
        concat_zeros = [
            np.zeros((N_CORES * z.shape[0], *z.shape[1:]), z.dtype) for z in zero_outs
        ]
        out_arrs = sharded(*concat_in, *concat_zeros)
        out_arrs = [a.block_until_ready() for a in out_arrs]
        return [
            {
                nm: np.asarray(out_arrs[i]).reshape(N_CORES, *out_avals[i].shape)[c]
                for i, nm in enumerate(out_names)
            }
            for c in range(N_CORES)
        ]

    def stage_async(in_maps, n_calls):
        from jax.sharding import NamedSharding

        sh = NamedSharding(mesh, PartitionSpec("core"))
        concat_in = [
            jax.device_put(
                np.concatenate([np.asarray(m[nm]) for m in in_maps], axis=0), sh
            )
            for nm in in_names
        ]
        zeros_sets = [
            [
                jax.device_put(
                    np.zeros((N_CORES * z.shape[0], *z.shape[1:]), z.dtype), sh
                )
                for z in zero_outs
            ]
            for _ in range(n_calls)
        ]
        for a in concat_in:
            a.block_until_ready()
        for zs in zeros_sets:
            for a in zs:
                a.block_until_ready()
        state = {"i": 0}

        def call(block=False):
            i = state["i"]
            state["i"] += 1
            outs = sharded(*concat_in, *zeros_sets[i])
            if block:
                outs = [np.asarray(a) for a in outs]
            return outs

        return call

    run.stage_async = stage_async
    _runner_cache[key] = run
    return run


N_CORES = 8
K = 12            # fp16 hi/lo split of the 6 quadratic-basis coefficients
STRIP_ROWS = 16
TILE_COLS = 16
F = STRIP_ROWS * TILE_COLS   # pixels per tile
OCT = 8                      # slots per octet (M = 3*OCT = 24 <= 30)
N_TILES = (256 // STRIP_ROWS) * (256 // TILE_COLS)
N_SLOTS = N_TILES // N_CORES
N_OCT = N_SLOTS // OCT
GROUP = 1536 // F            # chunks per fused Exp (3 PSUM banks)

_prog_cache = {}


def _octet_partition(profile):
    """Partition slot positions into N_OCT octets of OCT slots, minimizing
    total padded chunks sum(ceil(octet_sum/128)) (deterministic greedy +
    hill climb). Returns list of octets (each a list of slot indices)."""
    import random

    n = len(profile)
    order = sorted(range(n), key=lambda i: -profile[i])
    octs = [[] for _ in range(N_OCT)]
    sums = [0] * N_OCT
    for i in order:
        # least-filled octet with room
        cands = [o for o in range(N_OCT) if len(octs[o]) < OCT]
        o = min(cands, key=lambda o: sums[o])
        octs[o].append(i)
        sums[o] += profile[i]

    def cost(octs):
        # minimize total chunk padding; heavily penalize octets with <2
        # chunks (a 1-chunk octet shrinks the window in which the previous
        # same-bank octet's drain must complete before PE rewrites the bank)
        c = 0
        for o in octs:
            s = sum(profile[i] for i in o)
            ch = (s + 127) // 128
            c += 128 * ch - s
            if ch < 2:
                c += 10000
        return c

    rng = random.Random(0)
    cc = cost(octs)
    for _ in range(3000):
        a, b = rng.randrange(N_OCT), rng.randrange(N_OCT)
        if a == b:
            continue
        ia, ib = rng.randrange(OCT), rng.randrange(OCT)
        octs[a][ia], octs[b][ib] = octs[b][ib], octs[a][ia]
        c = cost(octs)
        if c <= cc:
            cc = c
        else:
            octs[a][ia], octs[b][ib] = octs[b][ib], octs[a][ia]
    return octs


def _structure(oct_sums):
    """Per-octet chunk counts and global chunk layout from octet gaussian
    sums. Returns (n_chunks_total, oct_chunks, oct_goff) where octet o's
    chunks span gaussians [oct_goff[o], oct_goff[o] + 128*oct_chunks[o]).
    Every octet gets at least one chunk so its output rows are written."""
    oct_chunks = [max(1, (s + 127) // 128) for s in oct_sums]
    oct_goff = [0]
    for c in oct_chunks:
        oct_goff.append(oct_goff[-1] + 128 * c)
    return sum(oct_chunks), oct_chunks, oct_goff


def _build_program(oct_chunks, repeat=1):
    """One SPMD program for the canonical octet chunk counts."""
    n_chunks = sum(oct_chunks)
    tot = n_chunks * 128
    M = 3 * OCT
    nc = bacc.Bacc(
        "TRN2",
        target_bir_lowering=False,
        debug=False,
        enable_asserts=True,
        num_devices=N_CORES,
    )
    f32, f16 = mybir.dt.float32, mybir.dt.float16
    # cb layout: basis first (cols [0, F)), then per-chunk coef columns
    cb_ext = nc.dram_tensor("cb", [K, F + tot], f16, kind="ExternalInput").ap()
    aux_ext = nc.dram_tensor("aux", [128, M * n_chunks], f16, kind="ExternalInput").ap()
    out_ext = nc.dram_tensor("out", [M, N_OCT * F], f16, kind="ExternalOutput").ap()
    head = F + min(GROUP * 128, tot)

    # chunk -> octet map and first/last flags
    ch_oct = []
    for o, c in enumerate(oct_chunks):
        ch_oct += [o] * c
    first_of = {}
    last_of = {}
    for j, o in enumerate(ch_oct):
        if o not in first_of:
            first_of[o] = j
        last_of[o] = j

    with tile.TileContext(nc) as tc:
        with (
            tc.tile_pool(name="consts", bufs=1) as consts,
            tc.tile_pool(name="work", bufs=3) as work,
            tc.tile_pool(name="outsb", bufs=1) as outsb,
            tc.tile_pool(name="psum", bufs=2, space="PSUM") as psum,
            tc.tile_pool(name="psum_acc", bufs=1, space="PSUM") as psum_acc,
        ):
            # trigger the exp ACT-table load immediately, concurrent with
            # the input DMAs
            dummy = consts.tile([128, 1], f32)
            zero_ap = nc.const_aps.aps[(f32, 0.0)]
            nc.scalar.activation(
                dummy[:], zero_ap, mybir.ActivationFunctionType.Exp, bias=0.0
            )

            cb_sb = consts.tile([44, F + tot], f16)
            # rows 0-11 and a duplicate at rows 32-43 (arg-matmul row-group
            # concurrency); both head-split so group 0 unblocks early, and on
            # DIFFERENT HWDGE queues (SP / Activation) so the two row groups
            # land in parallel — the first exp gates on both
            nc.sync.dma_start(out=cb_sb[0:K, :head], in_=cb_ext[:, :head])
            # the duplicate row group rides the idle GPSIMD (SWDGE) queue so
            # both copies land in parallel — the first exp gates on both
            nc.gpsimd.dma_start(out=cb_sb[32 : 32 + K, :head], in_=cb_ext[:, :head])
            aux_sb = consts.tile([128, M * n_chunks], f16)
            # colors are first needed after the first exp
            nc.scalar.dma_start(out=aux_sb[:], in_=aux_ext[:])
            if head < F + tot:
                nc.sync.dma_start(out=cb_sb[0:K, head:], in_=cb_ext[:, head:])
                nc.gpsimd.dma_start(out=cb_sb[32 : 32 + K, head:], in_=cb_ext[:, head:])

            out_sb = outsb.tile([M, N_OCT * F], f16)
            # two accumulator PSUM banks, octets alternating A/B in chunk
            # order: the drain of octet o (VectorE read) always overlaps PE
            # writes to the OTHER bank (PE-write + DVE-read of the same PSUM
            # bank is fatal on hardware). All color matmuls use rows 0..M-1
            # and tile_position (0,0): identical-range matmuls at different
            # tile positions get co-issued into one PE group by walrus and
            # hang the device, so keep every position identical.
            acc = [
                psum_acc.tile([M, 512], f32, tag=f"acc{b}", name=f"acc{b}")
                for b in range(2)
            ]

            def acc_region(o):
                c0 = (o // 2) * F
                return acc[o % 2][0:M, c0 : c0 + F]

            n_groups = (n_chunks + GROUP - 1) // GROUP
            group_js = [
                [j for j in range(g * GROUP, (g + 1) * GROUP) if j < n_chunks]
                for g in range(n_groups)
            ]

            # pixels-per-bank: chunks sharing one PSUM bank in the arg tile
            ppb = 512 // F

            def emit_args(js):
                arg_ps = psum.tile([128, GROUP * F], f32, tag="arg", name="arg_ps")
                # adjacent arg matmuls run CONCURRENTLY on alternating PE row
                # groups; two concurrent matmuls writing the same PSUM bank
                # hang the device. Emit in bank-interleaved order so adjacent
                # matmuls always target different banks.
                order = sorted(range(len(js)), key=lambda h: (h % ppb, h // ppb))
                for idx, h in enumerate(order):
                    j = js[h]
                    p = 32 * (idx % 2)
                    nc.tensor.matmul(
                        arg_ps[:, h * F : (h + 1) * F],
                        lhsT=cb_sb[p : p + K, F + j * 128 : F + (j + 1) * 128],
                        rhs=cb_sb[p : p + K, 0:F],
                        start=True,
                        stop=True,
                        tile_position=(p, 0),
                    )
                return arg_ps

            pend = None
            for rep in range(repeat):
                for gidx in range(n_groups):
                    js = group_js[gidx]
                    w = len(js) * F
                    arg_ps = pend if pend is not None else emit_args(js)
                    pend = None
                    alpha_sb = work.tile([128, GROUP * F], f16, tag="alpha")
                    nc.scalar.activation(
                        alpha_sb[:, :w],
                        arg_ps[:, :w],
                        mybir.ActivationFunctionType.Exp,
                        bias=0.0,
                    )
                    if gidx + 1 < n_groups:
                        pend = emit_args(group_js[gidx + 1])
                    elif rep < repeat - 1:
                        pend = emit_args(group_js[0])
                    for h, j in enumerate(js):
                        o = ch_oct[j]
                        nc.tensor.matmul(
                            acc_region(o),
                            lhsT=aux_sb[:, M * j : M * (j + 1)],
                            rhs=alpha_sb[:, h * F : (h + 1) * F],
                            start=(first_of[o] == j and rep == 0),
                            stop=(last_of[o] == j and rep == repeat - 1),
                            tile_position=(0, 0),
                        )
                        if rep == repeat - 1:
                            for o2 in range(N_OCT):
                                if last_of[o2] != j:
                                    continue
                                nc.vector.tensor_copy(
                                    out=out_sb[:, o2 * F : (o2 + 1) * F],
                                    in_=acc_region(o2),
                                )
                                nc.sync.dma_start(
                                    out=out_ext[:, o2 * F : (o2 + 1) * F],
                                    in_=out_sb[:, o2 * F : (o2 + 1) * F],
                                )
    nc.compile()
    return nc


def _get_program(oct_chunks, repeat=1):
    key = (tuple(oct_chunks), repeat)
    if key not in _prog_cache:
        _prog_cache[key] = _build_program(list(oct_chunks), repeat)
    return _prog_cache[key]


def _coefs(means, stds, rhos, cxo, cyo):
    """[6, G] coefficients of -0.5*mahal2 in local coords; f64 intermediates."""
    sx = stds[:, 0].astype(np.float64)
    sy = stds[:, 1].astype(np.float64)
    r = rhos.astype(np.float64)
    om = 1.0 - r * r
    ia = 1.0 / (sx * sx * om)
    ib = -r / (sx * sy * om)
    ic = 1.0 / (sy * sy * om)
    mxl = means[:, 0].astype(np.float64) - cxo
    myl = means[:, 1].astype(np.float64) - cyo
    return np.stack(
        [
            -0.5 * ia,
            -ib,
            -0.5 * ic,
            ia * mxl + ib * myl,
            ib * mxl + ic * myl,
            -0.5 * (ia * mxl * mxl + 2 * ib * mxl * myl + ic * myl * myl),
        ],
        axis=0,
    )


def _basis():
    ys = np.arange(STRIP_ROWS, dtype=np.float64) + 0.5 - STRIP_ROWS / 2
    xs = np.arange(TILE_COLS, dtype=np.float64) + 0.5 - TILE_COLS / 2
    yl = np.repeat(ys, TILE_COLS)
    xl = np.tile(xs, STRIP_ROWS)
    return np.stack(
        [xl * xl, xl * yl, yl * yl, xl, yl, np.ones_like(xl)], axis=0
    ).astype(np.float16)  # exact in fp16 at these tile extents


def kernel(
    opacity,
    means,
    stds,
    rhos,
    colors,
    image_height,
    image_width,
    scale_factor,
    raster_ratio,
    _repeat=1,
):
    H = int(image_height)
    W = int(image_width)
    sf = float(scale_factor)
    rr = float(raster_ratio)
    opacity = np.asarray(opacity, np.float32)
    means = np.asarray(means, np.float32)
    stds = np.asarray(stds, np.float32) * np.float32(sf)
    rhos = np.asarray(rhos, np.float32)
    colors = np.asarray(colors, np.float32)

    n_tiles_y = H // STRIP_ROWS
    n_tiles_x = W // TILE_COLS
    n_tiles = n_tiles_y * n_tiles_x
    assert n_tiles % N_CORES == 0
    n_slots = n_tiles // N_CORES
    assert n_slots == N_SLOTS

    # --- host-side cull: EXACT rr-sigma ellipse vs tile pixel-center box.
    # The reference zeroes weights beyond rr sigma, so a gaussian whose
    # minimum Mahalanobis distance over the tile's pixel-center box exceeds
    # rr contributes exactly nothing — excluding it adds no error.
    mx = means[:, 0].astype(np.float64)
    my = means[:, 1].astype(np.float64)
    _sx = stds[:, 0].astype(np.float64)
    _sy = stds[:, 1].astype(np.float64)
    _r = rhos.astype(np.float64)
    _om = 1.0 - _r * _r
    _ia = 1.0 / (_sx * _sx * _om)
    _ic = 1.0 / (_sy * _sy * _om)
    _ib = -_r / (_sx * _sy * _om)

    def _min_mahal2(x0, x1, y0, y1):
        """min of ia*dx^2 + 2*ib*dx*dy + ic*dy^2 over the box (vectorized)."""
        dx0, dx1 = x0 - mx, x1 - mx
        dy0, dy1 = y0 - my, y1 - my
        inside = (dx0 <= 0) & (dx1 >= 0) & (dy0 <= 0) & (dy1 >= 0)
        best = np.full(len(mx), np.inf)
        for a in (dx0, dx1):
            dys = np.clip(-_ib * a / _ic, dy0, dy1)
            best = np.minimum(best, _ia * a * a + 2 * _ib * a * dys + _ic * dys * dys)
        for b in (dy0, dy1):
            dxs = np.clip(-_ib * b / _ia, dx0, dx1)
            best = np.minimum(best, _ia * dxs * dxs + 2 * _ib * dxs * b + _ic * b * b)
        return np.where(inside, 0.0, best)

    tile_ids = []  # per tile: gaussian index array
    tile_pos = []  # per tile: (ty, tx) pixel origin
    for tyi in range(n_tiles_y):
        ty = tyi * STRIP_ROWS
        for txi in range(n_tiles_x):
            tx = txi * TILE_COLS
            m2 = _min_mahal2(tx + 0.5, tx + TILE_COLS - 0.5, ty + 0.5, ty + STRIP_ROWS - 0.5)
            tile_ids.append(np.nonzero(m2 <= rr * rr)[0])
            tile_pos.append((ty, tx))

    # snake-deal tiles to cores by descending count, so every core gets a
    # near-identical sorted profile (SPMD: slot capacity is the max over
    # cores at each slot position)
    counts = [len(ids) for ids in tile_ids]
    t_order = sorted(range(n_tiles), key=lambda t: -counts[t])
    assign = [[] for _ in range(N_CORES)]
    for i, t in enumerate(t_order):
        rnd, pos = divmod(i, N_CORES)
        core = pos if rnd % 2 == 0 else N_CORES - 1 - pos
        assign[core].append(t)
    profile = [
        max(counts[assign[core][k]] for core in range(N_CORES)) for k in range(n_slots)
    ]

    octs = _octet_partition(profile)
    oct_sums = [sum(profile[i] for i in o) for o in octs]
    n_chunks, oct_chunks, oct_goff = _structure(oct_sums)
    tot = n_chunks * 128
    M = 3 * OCT

    nc = _get_program(oct_chunks, _repeat)

    basis6 = _basis()  # [6, F] fp16, exact
    lnop_all = np.where(
        opacity > 0, np.log(np.maximum(opacity.astype(np.float64), 1e-45)), -1e4
    )

    # global gaussian offset of each slot (canonical): octet o's slots are
    # packed consecutively from oct_goff[o] in octs[o] order
    slot_goff = {}
    slot_octpos = {}
    for o, slot_list in enumerate(octs):
        g = oct_goff[o]
        for s_pos, k in enumerate(slot_list):
            slot_goff[k] = g
            slot_octpos[k] = (o, s_pos)
            g += profile[k]

    in_maps = []
    for core in range(N_CORES):
        chi = np.zeros((6, tot), np.float64)
        clo = np.zeros((6, tot), np.float64)
        chi[5, :] = -1e4
        colvals = np.zeros((tot, 3), np.float16)
        aux_arr = np.zeros((128, M * n_chunks), np.float16)
        for k in range(n_slots):
            t = assign[core][k]
            ty, tx = tile_pos[t]
            ids = tile_ids[t]
            g = len(ids)
            if not g:
                continue
            base = slot_goff[k]
            cf = _coefs(
                means[ids], stds[ids], rhos[ids],
                tx + TILE_COLS / 2, ty + STRIP_ROWS / 2,
            )
            cf[5] += lnop_all[ids]  # fold ln(opacity): exp needs no bias
            c_hi16 = cf.astype(np.float16)
            chi[:, base : base + g] = c_hi16
            clo[:, base : base + g] = cf - c_hi16.astype(np.float64)
            colvals[base : base + g] = colors[ids]
            # scatter colors into the per-chunk lhsT columns
            o, s_pos = slot_octpos[k]
            for r in range(base, base + g):
                j = r // 128
                aux_arr[r - j * 128, M * j + 3 * s_pos : M * j + 3 * s_pos + 3] = (
                    colvals[r]
                )
        cb_arr = np.zeros((K, F + tot), np.float16)
        cb_arr[0:6, 0:F] = basis6
        cb_arr[6:12, 0:F] = basis6
        cb_arr[0:6, F:] = chi.astype(np.float16)
        cb_arr[6:12, F:] = clo.astype(np.float16)
        in_maps.append({"cb": cb_arr, "aux": aux_arr})

    global _last_in_maps
    _last_in_maps = in_maps
    run = _get_runner(nc)
    results = run(in_maps)

    out = np.zeros((H, W, 3), np.float32)
    for core in range(N_CORES):
        o_arr = results[core]["out"].astype(np.float32)  # [M, N_OCT*F]
        for k in range(n_slots):
            o, s_pos = slot_octpos[k]
            ty, tx = tile_pos[assign[core][k]]
            blk = o_arr[3 * s_pos : 3 * s_pos + 3, o * F : (o + 1) * F].reshape(
                3, STRIP_ROWS, TILE_COLS
            )
            out[ty : ty + STRIP_ROWS, tx : tx + TILE_COLS, :] = blk.transpose(1, 2, 0)
    if _repeat > 1:
        out /= np.float32(_repeat)
    return out


# revision 18
# speedup vs baseline: 1.0065x; 1.0065x over previous
"""2D Gaussian splat rasterizer on 8 Trainium2 NeuronCores — v2.

Strategy: shard the image into tiles of SR x TC pixels (F = SR*TC pixels
per tile), n_slots tiles per core. Gaussians are culled host-side per
tile by their raster_ratio-sigma bounding box; per-tile counts (raw, no
quantization) form one canonical slot profile shared by all 8 cores
(SPMD). Slots are grouped into "octets" of OCT slots whose gaussians are
concatenated and padded to a multiple of 128 (the chunk size), so no
chunk crosses an octet boundary. Per chunk of 128 gaussians:

    arg   = coefT.T @ basis        TensorE, K=12 fp16 hi/lo split of the
                                   6-term pixel basis [x^2, xy, y^2, x, y, 1]
                                   (tile-local coords; ln(opacity) folded in)
    alpha = Exp(arg)               ScalarE, fused over a GROUP of chunks
                                   spanning 3 PSUM banks, fp16 out
    acc  += colorsT @ alpha        TensorE, ONE K=128 M=3*OCT matmul per
                                   chunk: each slot owns a fixed 3-row
                                   triplet in the octet's accumulator
                                   region; gaussians outside a slot get
                                   zero color columns, so arbitrary slot
                                   boundaries are fine (the quadratic form
                                   is <= 0 everywhere, so stray alphas are
                                   bounded by 1 and killed by zero colors)

Accumulator regions (24 rows x F cols) live in TWO PSUM banks with
octets alternating A/B in chunk order, so each drain (VectorE copy +
DMA, issued as soon as the octet's last chunk closes) overlaps PE writes
to the OTHER bank only — PE-write + DVE-read of one PSUM bank is fatal
on hardware. All color matmuls share one tile_position and full-K
partition ranges (identical-range matmuls at different tile positions
get co-issued into a conflicting PE group and hang the device), and arg
matmuls are emitted bank-interleaved so the row-group-concurrent
neighbors never write the same PSUM bank. Inputs arrive on three queues
(SP / Activation HWDGE + GPSIMD SWDGE) so both coef row groups land in
parallel; a dummy Exp pulls the ACT table load forward under the input
DMAs. Output is [24, n_oct*F] fp16 per core — only meaningful rows. The
full [H, W, 3] image is reassembled host-side (pure concatenation).
"""

import numpy as np
import concourse.bacc as bacc
import concourse.tile as tile
from concourse import mybir
from concourse.bass_utils import run_bass_kernel_spmd

_runner_cache = {}


def _get_runner(nc):
    """Persistent jitted SPMD executor for a compiled Bass program."""
    key = id(nc)
    if key in _runner_cache:
        return _runner_cache[key]
    import jax
    from jax.sharding import Mesh, PartitionSpec
    from jax.experimental.shard_map import shard_map
    from concourse import bass2jax, mybir as mb

    bass2jax.install_neuronx_cc_hook()

    in_names, out_names, out_avals, zero_outs = [], [], [], []
    partition_name = nc.partition_id_tensor.name if nc.partition_id_tensor else None
    for alloc in nc.m.functions[0].allocations:
        if not isinstance(alloc, mb.MemoryLocationSet):
            continue
        name = alloc.memorylocations[0].name
        if alloc.kind == "ExternalInput":
            if name != partition_name:
                in_names.append(name)
        elif alloc.kind == "ExternalOutput":
            shape = tuple(alloc.tensor_shape)
            dtype = mb.dt.np(alloc.dtype)
            out_names.append(name)
            out_avals.append(jax.core.ShapedArray(shape, dtype))
            zero_outs.append(np.zeros(shape, dtype))
    n_params = len(in_names)
    all_in = in_names + out_names + ([partition_name] if partition_name else [])

    def _body(*args):
        operands = list(args)
        if partition_name is not None:
            operands.append(bass2jax.partition_id_tensor())
        outs = bass2jax._bass_exec_p.bind(
            *operands,
            out_avals=tuple(out_avals),
            in_names=tuple(all_in),
            out_names=tuple(out_names),
            lowering_input_output_aliases=(),
            sim_require_finite=True,
            sim_require_nnan=True,
            nc=nc,
        )
        return tuple(outs)

    devices = jax.devices()[:N_CORES]
    mesh = Mesh(np.asarray(devices), ("core",))
    in_specs = (PartitionSpec("core"),) * (n_params + len(out_names))
    out_specs = (PartitionSpec("core"),) * len(out_names)
    sharded = jax.jit(
        shard_map(
            _body, mesh=mesh, in_specs=in_specs, out_specs=out_specs, check_rep=False
        ),
        donate_argnums=tuple(range(n_params, n_params + len(out_names))),
        keep_unused=True,
    )

    def run(in_maps):
        concat_in = [
            np.concatenate([np.asarray(m[nm]) for m in in_maps], axis=0)
            for nm in in_names
        ]
        concat_zeros = [
            np.zeros((N_CORES * z.shape[0], *z.shape[1:]), z.dtype) for z in zero_outs
        ]
        out_arrs = sharded(*concat_in, *concat_zeros)
        out_arrs = [a.block_until_ready() for a in out_arrs]
        return [
            {
                nm: np.asarray(out_arrs[i]).reshape(N_CORES, *out_avals[i].shape)[c]
                for i, nm in enumerate(out_names)
            }
            for c in range(N_CORES)
        ]

    def stage_async(in_maps, n_calls):
        from jax.sharding import NamedSharding

        sh = NamedSharding(mesh, PartitionSpec("core"))
        concat_in = [
            jax.device_put(
                np.concatenate([np.asarray(m[nm]) for m in in_maps], axis=0), sh
            )
            for nm in in_names
        ]
        zeros_sets = [
            [
                jax.device_put(
                    np.zeros((N_CORES * z.shape[0], *z.shape[1:]), z.dtype), sh
                )
                for z in zero_outs
            ]
            for _ in range(n_calls)
        ]
        for a in concat_in:
            a.block_until_ready()
        for zs in zeros_sets:
            for a in zs:
                a.block_until_ready()
        state = {"i": 0}

        def call(block=False):
            i = state["i"]
            state["i"] += 1
            outs = sharded(*concat_in, *zeros_sets[i])
            if block:
                outs = [np.asarray(a) for a in outs]
            return outs

        return call

    run.stage_async = stage_async
    _runner_cache[key] = run
    return run


N_CORES = 8
K = 12            # fp16 hi/lo split of the 6 quadratic-basis coefficients
STRIP_ROWS = 16
TILE_COLS = 16
F = STRIP_ROWS * TILE_COLS   # pixels per tile
OCT = 8                      # slots per octet (M = 3*OCT = 24 <= 30)
N_TILES = (256 // STRIP_ROWS) * (256 // TILE_COLS)
N_SLOTS = N_TILES // N_CORES
N_OCT = N_SLOTS // OCT
GROUP = 1536 // F            # chunks per fused Exp (3 PSUM banks)

_prog_cache = {}


def _octet_partition(profile):
    """Partition slot positions into N_OCT octets of OCT slots, minimizing
    total padded chunks sum(ceil(octet_sum/128)) (deterministic greedy +
    hill climb). Returns list of octets (each a list of slot indices)."""
    import random

    n = len(profile)
    order = sorted(range(n), key=lambda i: -profile[i])
    octs = [[] for _ in range(N_OCT)]
    sums = [0] * N_OCT
    for i in order:
        # least-filled octet with room
        cands = [o for o in range(N_OCT) if len(octs[o]) < OCT]
        o = min(cands, key=lambda o: sums[o])
        octs[o].append(i)
        sums[o] += profile[i]

    def cost(octs):
        # minimize total chunk padding; heavily penalize octets with <2
        # chunks (a 1-chunk octet shrinks the window in which the previous
        # same-bank octet's drain must complete before PE rewrites the bank)
        c = 0
        for o in octs:
            s = sum(profile[i] for i in o)
            ch = (s + 127) // 128
            c += 128 * ch - s
            if ch < 2:
                c += 10000
        return c

    rng = random.Random(0)
    cc = cost(octs)
    for _ in range(3000):
        a, b = rng.randrange(N_OCT), rng.randrange(N_OCT)
        if a == b:
            continue
        ia, ib = rng.randrange(OCT), rng.randrange(OCT)
        octs[a][ia], octs[b][ib] = octs[b][ib], octs[a][ia]
        c = cost(octs)
        if c <= cc:
            cc = c
        else:
            octs[a][ia], octs[b][ib] = octs[b][ib], octs[a][ia]
    return octs


def _structure(oct_sums):
    """Per-octet chunk counts and global chunk layout from octet gaussian
    sums. Returns (n_chunks_total, oct_chunks, oct_goff) where octet o's
    chunks span gaussians [oct_goff[o], oct_goff[o] + 128*oct_chunks[o]).
    Every octet gets at least one chunk so its output rows are written."""
    oct_chunks = [max(1, (s + 127) // 128) for s in oct_sums]
    oct_goff = [0]
    for c in oct_chunks:
        oct_goff.append(oct_goff[-1] + 128 * c)
    return sum(oct_chunks), oct_chunks, oct_goff


def _build_program(oct_chunks, repeat=1):
    """One SPMD program for the canonical octet chunk counts."""
    n_chunks = sum(oct_chunks)
    tot = n_chunks * 128
    M = 3 * OCT
    nc = bacc.Bacc(
        "TRN2",
        target_bir_lowering=False,
        debug=False,
        enable_asserts=True,
        num_devices=N_CORES,
    )
    f32, f16 = mybir.dt.float32, mybir.dt.float16
    # cb layout: basis first (cols [0, F)), then per-chunk coef columns
    cb_ext = nc.dram_tensor("cb", [K, F + tot], f16, kind="ExternalInput").ap()
    aux_ext = nc.dram_tensor("aux", [128, M * n_chunks], f16, kind="ExternalInput").ap()
    out_ext = nc.dram_tensor("out", [M, N_OCT * F], f16, kind="ExternalOutput").ap()
    head = F + min(GROUP * 128, tot)

    # chunk -> octet map and first/last flags
    ch_oct = []
    for o, c in enumerate(oct_chunks):
        ch_oct += [o] * c
    first_of = {}
    last_of = {}
    for j, o in enumerate(ch_oct):
        if o not in first_of:
            first_of[o] = j
        last_of[o] = j

    with tile.TileContext(nc) as tc:
        with (
            tc.tile_pool(name="consts", bufs=1) as consts,
            tc.tile_pool(name="work", bufs=3) as work,
            tc.tile_pool(name="outsb", bufs=1) as outsb,
            tc.tile_pool(name="psum", bufs=2, space="PSUM") as psum,
            tc.tile_pool(name="psum_acc", bufs=1, space="PSUM") as psum_acc,
        ):
            # trigger the exp ACT-table load immediately, concurrent with
            # the input DMAs
            dummy = consts.tile([128, 1], f32)
            zero_ap = nc.const_aps.aps[(f32, 0.0)]
            nc.scalar.activation(
                dummy[:], zero_ap, mybir.ActivationFunctionType.Exp, bias=0.0
            )

            cb_sb = consts.tile([44, F + tot], f16)
            # rows 0-11 and a duplicate at rows 32-43 (arg-matmul row-group
            # concurrency), on different queues (SP HWDGE / GPSIMD SWDGE) so
            # both copies land in parallel. One DMA each: the transfer itself
            # is tiny, so splitting only adds queue-dispatch serialization.
            nc.sync.dma_start(out=cb_sb[0:K, :], in_=cb_ext[:, :])
            nc.gpsimd.dma_start(out=cb_sb[32 : 32 + K, :], in_=cb_ext[:, :])
            aux_sb = consts.tile([128, M * n_chunks], f16)
            # colors are first needed after the first exp
            nc.scalar.dma_start(out=aux_sb[:], in_=aux_ext[:])

            out_sb = outsb.tile([M, N_OCT * F], f16)
            # two accumulator PSUM banks, octets alternating A/B in chunk
            # order: the drain of octet o (VectorE read) always overlaps PE
            # writes to the OTHER bank (PE-write + DVE-read of the same PSUM
            # bank is fatal on hardware). All color matmuls use rows 0..M-1
            # and tile_position (0,0): identical-range matmuls at different
            # tile positions get co-issued into one PE group by walrus and
            # hang the device, so keep every position identical.
            acc = [
                psum_acc.tile([M, 512], f32, tag=f"acc{b}", name=f"acc{b}")
                for b in range(2)
            ]

            def acc_region(o):
                c0 = (o // 2) * F
                return acc[o % 2][0:M, c0 : c0 + F]

            n_groups = (n_chunks + GROUP - 1) // GROUP
            group_js = [
                [j for j in range(g * GROUP, (g + 1) * GROUP) if j < n_chunks]
                for g in range(n_groups)
            ]

            # pixels-per-bank: chunks sharing one PSUM bank in the arg tile
            ppb = 512 // F

            def emit_args(js, single_rowgroup=False):
                arg_ps = psum.tile([128, GROUP * F], f32, tag="arg", name="arg_ps")
                # adjacent arg matmuls run CONCURRENTLY on alternating PE row
                # groups; two concurrent matmuls writing the same PSUM bank
                # hang the device. Emit in bank-interleaved order so adjacent
                # matmuls always target different banks. The FIRST group uses
                # a single row group (p=0, serial — same-position matmuls
                # never co-issue) so the first exp gates only on the earliest
                # input DMA, not on the duplicate row-group copy.
                order = sorted(range(len(js)), key=lambda h: (h % ppb, h // ppb))
                for idx, h in enumerate(order):
                    j = js[h]
                    p = 0 if single_rowgroup else 32 * (idx % 2)
                    nc.tensor.matmul(
                        arg_ps[:, h * F : (h + 1) * F],
                        lhsT=cb_sb[p : p + K, F + j * 128 : F + (j + 1) * 128],
                        rhs=cb_sb[p : p + K, 0:F],
                        start=True,
                        stop=True,
                        tile_position=(p, 0),
                    )
                return arg_ps

            pend = None
            for rep in range(repeat):
                for gidx in range(n_groups):
                    js = group_js[gidx]
                    w = len(js) * F
                    arg_ps = (
                        pend
                        if pend is not None
                        else emit_args(js, single_rowgroup=(rep == 0 and gidx == 0))
                    )
                    pend = None
                    alpha_sb = work.tile([128, GROUP * F], f16, tag="alpha")
                    nc.scalar.activation(
                        alpha_sb[:, :w],
                        arg_ps[:, :w],
                        mybir.ActivationFunctionType.Exp,
                        bias=0.0,
                    )
                    if gidx + 1 < n_groups:
                        pend = emit_args(group_js[gidx + 1])
                    elif rep < repeat - 1:
                        pend = emit_args(group_js[0])
                    for h, j in enumerate(js):
                        o = ch_oct[j]
                        nc.tensor.matmul(
                            acc_region(o),
                            lhsT=aux_sb[:, M * j : M * (j + 1)],
                            rhs=alpha_sb[:, h * F : (h + 1) * F],
                            start=(first_of[o] == j and rep == 0),
                            stop=(last_of[o] == j and rep == repeat - 1),
                            tile_position=(0, 0),
                        )
                        if rep == repeat - 1:
                            for o2 in range(N_OCT):
                                if last_of[o2] != j:
                                    continue
                                nc.vector.tensor_copy(
                                    out=out_sb[:, o2 * F : (o2 + 1) * F],
                                    in_=acc_region(o2),
                                )
                                nc.sync.dma_start(
                                    out=out_ext[:, o2 * F : (o2 + 1) * F],
                                    in_=out_sb[:, o2 * F : (o2 + 1) * F],
                                )
    nc.compile()
    return nc


def _get_program(oct_chunks, repeat=1):
    key = (tuple(oct_chunks), repeat)
    if key not in _prog_cache:
        _prog_cache[key] = _build_program(list(oct_chunks), repeat)
    return _prog_cache[key]


def _coefs(means, stds, rhos, cxo, cyo):
    """[6, G] coefficients of -0.5*mahal2 in local coords; f64 intermediates."""
    sx = stds[:, 0].astype(np.float64)
    sy = stds[:, 1].astype(np.float64)
    r = rhos.astype(np.float64)
    om = 1.0 - r * r
    ia = 1.0 / (sx * sx * om)
    ib = -r / (sx * sy * om)
    ic = 1.0 / (sy * sy * om)
    mxl = means[:, 0].astype(np.float64) - cxo
    myl = means[:, 1].astype(np.float64) - cyo
    return np.stack(
        [
            -0.5 * ia,
            -ib,
            -0.5 * ic,
            ia * mxl + ib * myl,
            ib * mxl + ic * myl,
            -0.5 * (ia * mxl * mxl + 2 * ib * mxl * myl + ic * myl * myl),
        ],
        axis=0,
    )


def _basis():
    ys = np.arange(STRIP_ROWS, dtype=np.float64) + 0.5 - STRIP_ROWS / 2
    xs = np.arange(TILE_COLS, dtype=np.float64) + 0.5 - TILE_COLS / 2
    yl = np.repeat(ys, TILE_COLS)
    xl = np.tile(xs, STRIP_ROWS)
    return np.stack(
        [xl * xl, xl * yl, yl * yl, xl, yl, np.ones_like(xl)], axis=0
    ).astype(np.float16)  # exact in fp16 at these tile extents


def kernel(
    opacity,
    means,
    stds,
    rhos,
    colors,
    image_height,
    image_width,
    scale_factor,
    raster_ratio,
    _repeat=1,
):
    H = int(image_height)
    W = int(image_width)
    sf = float(scale_factor)
    rr = float(raster_ratio)
    opacity = np.asarray(opacity, np.float32)
    means = np.asarray(means, np.float32)
    stds = np.asarray(stds, np.float32) * np.float32(sf)
    rhos = np.asarray(rhos, np.float32)
    colors = np.asarray(colors, np.float32)

    n_tiles_y = H // STRIP_ROWS
    n_tiles_x = W // TILE_COLS
    n_tiles = n_tiles_y * n_tiles_x
    assert n_tiles % N_CORES == 0
    n_slots = n_tiles // N_CORES
    assert n_slots == N_SLOTS

    # --- host-side cull: EXACT rr-sigma ellipse vs tile pixel-center box.
    # The reference zeroes weights beyond rr sigma, so a gaussian whose
    # minimum Mahalanobis distance over the tile's pixel-center box exceeds
    # rr contributes exactly nothing — excluding it adds no error.
    mx = means[:, 0].astype(np.float64)
    my = means[:, 1].astype(np.float64)
    _sx = stds[:, 0].astype(np.float64)
    _sy = stds[:, 1].astype(np.float64)
    _r = rhos.astype(np.float64)
    _om = 1.0 - _r * _r
    _ia = 1.0 / (_sx * _sx * _om)
    _ic = 1.0 / (_sy * _sy * _om)
    _ib = -_r / (_sx * _sy * _om)

    def _min_mahal2(x0, x1, y0, y1):
        """min of ia*dx^2 + 2*ib*dx*dy + ic*dy^2 over the box (vectorized)."""
        dx0, dx1 = x0 - mx, x1 - mx
        dy0, dy1 = y0 - my, y1 - my
        inside = (dx0 <= 0) & (dx1 >= 0) & (dy0 <= 0) & (dy1 >= 0)
        best = np.full(len(mx), np.inf)
        for a in (dx0, dx1):
            dys = np.clip(-_ib * a / _ic, dy0, dy1)
            best = np.minimum(best, _ia * a * a + 2 * _ib * a * dys + _ic * dys * dys)
        for b in (dy0, dy1):
            dxs = np.clip(-_ib * b / _ia, dx0, dx1)
            best = np.minimum(best, _ia * dxs * dxs + 2 * _ib * dxs * b + _ic * b * b)
        return np.where(inside, 0.0, best)

    tile_ids = []  # per tile: gaussian index array
    tile_pos = []  # per tile: (ty, tx) pixel origin
    for tyi in range(n_tiles_y):
        ty = tyi * STRIP_ROWS
        for txi in range(n_tiles_x):
            tx = txi * TILE_COLS
            m2 = _min_mahal2(tx + 0.5, tx + TILE_COLS - 0.5, ty + 0.5, ty + STRIP_ROWS - 0.5)
            tile_ids.append(np.nonzero(m2 <= rr * rr)[0])
            tile_pos.append((ty, tx))

    # snake-deal tiles to cores by descending count, so every core gets a
    # near-identical sorted profile (SPMD: slot capacity is the max over
    # cores at each slot position)
    counts = [len(ids) for ids in tile_ids]
    t_order = sorted(range(n_tiles), key=lambda t: -counts[t])
    assign = [[] for _ in range(N_CORES)]
    for i, t in enumerate(t_order):
        rnd, pos = divmod(i, N_CORES)
        core = pos if rnd % 2 == 0 else N_CORES - 1 - pos
        assign[core].append(t)
    profile = [
        max(counts[assign[core][k]] for core in range(N_CORES)) for k in range(n_slots)
    ]

    octs = _octet_partition(profile)
    oct_sums = [sum(profile[i] for i in o) for o in octs]
    n_chunks, oct_chunks, oct_goff = _structure(oct_sums)
    tot = n_chunks * 128
    M = 3 * OCT

    nc = _get_program(oct_chunks, _repeat)

    basis6 = _basis()  # [6, F] fp16, exact
    lnop_all = np.where(
        opacity > 0, np.log(np.maximum(opacity.astype(np.float64), 1e-45)), -1e4
    )

    # global gaussian offset of each slot (canonical): octet o's slots are
    # packed consecutively from oct_goff[o] in octs[o] order
    slot_goff = {}
    slot_octpos = {}
    for o, slot_list in enumerate(octs):
        g = oct_goff[o]
        for s_pos, k in enumerate(slot_list):
            slot_goff[k] = g
            slot_octpos[k] = (o, s_pos)
            g += profile[k]

    in_maps = []
    for core in range(N_CORES):
        chi = np.zeros((6, tot), np.float64)
        clo = np.zeros((6, tot), np.float64)
        chi[5, :] = -1e4
        colvals = np.zeros((tot, 3), np.float16)
        aux_arr = np.zeros((128, M * n_chunks), np.float16)
        for k in range(n_slots):
            t = assign[core][k]
            ty, tx = tile_pos[t]
            ids = tile_ids[t]
            g = len(ids)
            if not g:
                continue
            base = slot_goff[k]
            cf = _coefs(
                means[ids], stds[ids], rhos[ids],
                tx + TILE_COLS / 2, ty + STRIP_ROWS / 2,
            )
            cf[5] += lnop_all[ids]  # fold ln(opacity): exp needs no bias
            c_hi16 = cf.astype(np.float16)
            chi[:, base : base + g] = c_hi16
            clo[:, base : base + g] = cf - c_hi16.astype(np.float64)
            colvals[base : base + g] = colors[ids]
            # scatter colors into the per-chunk lhsT columns
            o, s_pos = slot_octpos[k]
            for r in range(base, base + g):
                j = r // 128
                aux_arr[r - j * 128, M * j + 3 * s_pos : M * j + 3 * s_pos + 3] = (
                    colvals[r]
                )
        cb_arr = np.zeros((K, F + tot), np.float16)
        cb_arr[0:6, 0:F] = basis6
        cb_arr[6:12, 0:F] = basis6
        cb_arr[0:6, F:] = chi.astype(np.float16)
        cb_arr[6:12, F:] = clo.astype(np.float16)
        in_maps.append({"cb": cb_arr, "aux": aux_arr})

    global _last_in_maps
    _last_in_maps = in_maps
    run = _get_runner(nc)
    results = run(in_maps)

    out = np.zeros((H, W, 3), np.float32)
    for core in range(N_CORES):
        o_arr = results[core]["out"].astype(np.float32)  # [M, N_OCT*F]
        for k in range(n_slots):
            o, s_pos = slot_octpos[k]
            ty, tx = tile_pos[assign[core][k]]
            blk = o_arr[3 * s_pos : 3 * s_pos + 3, o * F : (o + 1) * F].reshape(
                3, STRIP_ROWS, TILE_COLS
            )
            out[ty : ty + STRIP_ROWS, tx : tx + TILE_COLS, :] = blk.transpose(1, 2, 0)
    if _repeat > 1:
        out /= np.float32(_repeat)
    return out


# revision 27
# speedup vs baseline: 1.0078x; 1.0013x over previous
"""2D Gaussian splat rasterizer on 8 Trainium2 NeuronCores — v2.

Strategy: shard the image into tiles of SR x TC pixels (F = SR*TC pixels
per tile), n_slots tiles per core. Gaussians are culled host-side per
tile by their raster_ratio-sigma bounding box; per-tile counts (raw, no
quantization) form one canonical slot profile shared by all 8 cores
(SPMD). Slots are grouped into "octets" of OCT slots whose gaussians are
concatenated and padded to a multiple of 128 (the chunk size), so no
chunk crosses an octet boundary. Per chunk of 128 gaussians:

    arg   = coefT.T @ basis        TensorE, K=12 fp16 hi/lo split of the
                                   6-term pixel basis [x^2, xy, y^2, x, y, 1]
                                   (tile-local coords; ln(opacity) folded in)
    alpha = Exp(arg)               ScalarE, fused over a GROUP of chunks
                                   spanning 3 PSUM banks, fp16 out
    acc  += colorsT @ alpha        TensorE, ONE K=128 M=3*OCT matmul per
                                   chunk: each slot owns a fixed 3-row
                                   triplet in the octet's accumulator
                                   region; gaussians outside a slot get
                                   zero color columns, so arbitrary slot
                                   boundaries are fine (the quadratic form
                                   is <= 0 everywhere, so stray alphas are
                                   bounded by 1 and killed by zero colors)

Accumulator regions (24 rows x F cols) live in TWO PSUM banks with
octets alternating A/B in chunk order, so each drain (VectorE copy +
DMA, issued as soon as the octet's last chunk closes) overlaps PE writes
to the OTHER bank only — PE-write + DVE-read of one PSUM bank is fatal
on hardware. All color matmuls share one tile_position and full-K
partition ranges (identical-range matmuls at different tile positions
get co-issued into a conflicting PE group and hang the device), and arg
matmuls are emitted bank-interleaved so the row-group-concurrent
neighbors never write the same PSUM bank. Inputs arrive on three queues
(SP / Activation HWDGE + GPSIMD SWDGE) so both coef row groups land in
parallel; a dummy Exp pulls the ACT table load forward under the input
DMAs. Output is [24, n_oct*F] fp16 per core — only meaningful rows. The
full [H, W, 3] image is reassembled host-side (pure concatenation).
"""

import numpy as np
import concourse.bacc as bacc
import concourse.tile as tile
from concourse import mybir
from concourse.bass_utils import run_bass_kernel_spmd

_runner_cache = {}


def _get_runner(nc):
    """Persistent jitted SPMD executor for a compiled Bass program."""
    key = id(nc)
    if key in _runner_cache:
        return _runner_cache[key]
    import jax
    from jax.sharding import Mesh, PartitionSpec
    from jax.experimental.shard_map import shard_map
    from concourse import bass2jax, mybir as mb

    bass2jax.install_neuronx_cc_hook()

    in_names, out_names, out_avals, zero_outs = [], [], [], []
    partition_name = nc.partition_id_tensor.name if nc.partition_id_tensor else None
    for alloc in nc.m.functions[0].allocations:
        if not isinstance(alloc, mb.MemoryLocationSet):
            continue
        name = alloc.memorylocations[0].name
        if alloc.kind == "ExternalInput":
            if name != partition_name:
                in_names.append(name)
        elif alloc.kind == "ExternalOutput":
            shape = tuple(alloc.tensor_shape)
            dtype = mb.dt.np(alloc.dtype)
            out_names.append(name)
            out_avals.append(jax.core.ShapedArray(shape, dtype))
            zero_outs.append(np.zeros(shape, dtype))
    n_params = len(in_names)
    all_in = in_names + out_names + ([partition_name] if partition_name else [])

    def _body(*args):
        operands = list(args)
        if partition_name is not None:
            operands.append(bass2jax.partition_id_tensor())
        outs = bass2jax._bass_exec_p.bind(
            *operands,
            out_avals=tuple(out_avals),
            in_names=tuple(all_in),
            out_names=tuple(out_names),
            lowering_input_output_aliases=(),
            sim_require_finite=True,
            sim_require_nnan=True,
            nc=nc,
        )
        return tuple(outs)

    devices = jax.devices()[:N_CORES]
    mesh = Mesh(np.asarray(devices), ("core",))
    in_specs = (PartitionSpec("core"),) * (n_params + len(out_names))
    out_specs = (PartitionSpec("core"),) * len(out_names)
    sharded = jax.jit(
        shard_map(
            _body, mesh=mesh, in_specs=in_specs, out_specs=out_specs, check_rep=False
        ),
        donate_argnums=tuple(range(n_params, n_params + len(out_names))),
        keep_unused=True,
    )

    def run(in_maps):
        concat_in = [
            np.concatenate([np.asarray(m[nm]) for m in in_maps], axis=0)
            for nm in in_names
        ]
        concat_zeros = [
            np.zeros((N_CORES * z.shape[0], *z.shape[1:]), z.dtype) for z in zero_outs
        ]
        out_arrs = sharded(*concat_in, *concat_zeros)
        out_arrs = [a.block_until_ready() for a in out_arrs]
        return [
            {
                nm: np.asarray(out_arrs[i]).reshape(N_CORES, *out_avals[i].shape)[c]
                for i, nm in enumerate(out_names)
            }
            for c in range(N_CORES)
        ]

    def stage_async(in_maps, n_calls):
        from jax.sharding import NamedSharding

        sh = NamedSharding(mesh, PartitionSpec("core"))
        concat_in = [
            jax.device_put(
                np.concatenate([np.asarray(m[nm]) for m in in_maps], axis=0), sh
            )
            for nm in in_names
        ]
        zeros_sets = [
            [
                jax.device_put(
                    np.zeros((N_CORES * z.shape[0], *z.shape[1:]), z.dtype), sh
                )
                for z in zero_outs
            ]
            for _ in range(n_calls)
        ]
        for a in concat_in:
            a.block_until_ready()
        for zs in zeros_sets:
            for a in zs:
                a.block_until_ready()
        state = {"i": 0}

        def call(block=False):
            i = state["i"]
            state["i"] += 1
            outs = sharded(*concat_in, *zeros_sets[i])
            if block:
                outs = [np.asarray(a) for a in outs]
            return outs

        return call

    run.stage_async = stage_async
    _runner_cache[key] = run
    return run


N_CORES = 8
K = 12            # fp16 hi/lo split of the 6 quadratic-basis coefficients
STRIP_ROWS = 16
TILE_COLS = 16
F = STRIP_ROWS * TILE_COLS   # pixels per tile
OCT = 8                      # slots per octet (M = 3*OCT = 24 <= 30)
N_TILES = (256 // STRIP_ROWS) * (256 // TILE_COLS)
N_SLOTS = N_TILES // N_CORES
N_OCT = N_SLOTS // OCT
GROUP = 1536 // F            # chunks per fused Exp (3 PSUM banks)

_prog_cache = {}


def _octet_partition(profile):
    """Partition slot positions into N_OCT octets of OCT slots, minimizing
    total padded chunks sum(ceil(octet_sum/128)) (deterministic greedy +
    hill climb). Returns list of octets (each a list of slot indices)."""
    import random

    n = len(profile)
    order = sorted(range(n), key=lambda i: -profile[i])
    octs = [[] for _ in range(N_OCT)]
    sums = [0] * N_OCT
    for i in order:
        # least-filled octet with room
        cands = [o for o in range(N_OCT) if len(octs[o]) < OCT]
        o = min(cands, key=lambda o: sums[o])
        octs[o].append(i)
        sums[o] += profile[i]

    def cost(octs):
        # minimize total chunk padding; then balance chunk counts — an
        # octet with few chunks shrinks the window in which the previous
        # same-bank octet's drain must complete before PE rewrites the bank
        c = 0
        chs = []
        for o in octs:
            s = sum(profile[i] for i in o)
            ch = (s + 127) // 128
            chs.append(ch)
            c += 128 * ch - s
            if ch < 2:
                c += 10000
        return c + 8 * (max(chs) - min(chs))

    rng = random.Random(0)
    cc = cost(octs)
    for _ in range(3000):
        a, b = rng.randrange(N_OCT), rng.randrange(N_OCT)
        if a == b:
            continue
        ia, ib = rng.randrange(OCT), rng.randrange(OCT)
        octs[a][ia], octs[b][ib] = octs[b][ib], octs[a][ia]
        c = cost(octs)
        if c <= cc:
            cc = c
        else:
            octs[a][ia], octs[b][ib] = octs[b][ib], octs[a][ia]
    return octs


def _structure(oct_sums):
    """Per-octet chunk counts and global chunk layout from octet gaussian
    sums. Returns (n_chunks_total, oct_chunks, oct_goff) where octet o's
    chunks span gaussians [oct_goff[o], oct_goff[o] + 128*oct_chunks[o]).
    Every octet gets at least one chunk so its output rows are written."""
    oct_chunks = [max(1, (s + 127) // 128) for s in oct_sums]
    oct_goff = [0]
    for c in oct_chunks:
        oct_goff.append(oct_goff[-1] + 128 * c)
    return sum(oct_chunks), oct_chunks, oct_goff


def _build_program(oct_chunks, repeat=1):
    """One SPMD program for the canonical octet chunk counts."""
    n_chunks = sum(oct_chunks)
    tot = n_chunks * 128
    M = 3 * OCT
    nc = bacc.Bacc(
        "TRN2",
        target_bir_lowering=False,
        debug=False,
        enable_asserts=True,
        num_devices=N_CORES,
    )
    f32, f16 = mybir.dt.float32, mybir.dt.float16
    # cb layout: basis first (cols [0, F)), then per-chunk coef columns
    cb_ext = nc.dram_tensor("cb", [K, F + tot], f16, kind="ExternalInput").ap()
    aux_ext = nc.dram_tensor("aux", [128, M * n_chunks], f16, kind="ExternalInput").ap()
    out_ext = nc.dram_tensor("out", [M, N_OCT * F], f16, kind="ExternalOutput").ap()
    head = F + min(GROUP * 128, tot)

    # chunk -> octet map and first/last flags
    ch_oct = []
    for o, c in enumerate(oct_chunks):
        ch_oct += [o] * c
    first_of = {}
    last_of = {}
    for j, o in enumerate(ch_oct):
        if o not in first_of:
            first_of[o] = j
        last_of[o] = j

    with tile.TileContext(nc) as tc:
        with (
            tc.tile_pool(name="consts", bufs=1) as consts,
            tc.tile_pool(name="work", bufs=3) as work,
            tc.tile_pool(name="outsb", bufs=1) as outsb,
            tc.tile_pool(name="psum", bufs=2, space="PSUM") as psum,
            tc.tile_pool(name="psum_acc", bufs=1, space="PSUM") as psum_acc,
        ):
            # trigger the exp ACT-table load immediately, concurrent with
            # the input DMAs
            dummy = consts.tile([128, 1], f32)
            zero_ap = nc.const_aps.aps[(f32, 0.0)]
            nc.scalar.activation(
                dummy[:], zero_ap, mybir.ActivationFunctionType.Exp, bias=0.0
            )

            cb_sb = consts.tile([44, F + tot], f16)
            # rows 0-11 and a duplicate at rows 32-43 (arg-matmul row-group
            # concurrency), on different queues (SP HWDGE / GPSIMD SWDGE) so
            # both copies land in parallel. One DMA each: the transfer itself
            # is tiny, so splitting only adds queue-dispatch serialization.
            nc.sync.dma_start(out=cb_sb[0:K, :], in_=cb_ext[:, :])
            nc.gpsimd.dma_start(out=cb_sb[32 : 32 + K, :], in_=cb_ext[:, :])
            aux_sb = consts.tile([128, M * n_chunks], f16)
            # colors are first needed after the first exp
            nc.scalar.dma_start(out=aux_sb[:], in_=aux_ext[:])

            out_sb = outsb.tile([M, N_OCT * F], f16)
            # two accumulator PSUM banks, octets alternating A/B in chunk
            # order: the drain of octet o (VectorE read) always overlaps PE
            # writes to the OTHER bank (PE-write + DVE-read of the same PSUM
            # bank is fatal on hardware). All color matmuls use rows 0..M-1
            # and tile_position (0,0): identical-range matmuls at different
            # tile positions get co-issued into one PE group by walrus and
            # hang the device, so keep every position identical.
            acc = [
                psum_acc.tile([M, 512], f32, tag=f"acc{b}", name=f"acc{b}")
                for b in range(2)
            ]

            def acc_region(o):
                c0 = (o // 2) * F
                return acc[o % 2][0:M, c0 : c0 + F]

            n_groups = (n_chunks + GROUP - 1) // GROUP
            group_js = [
                [j for j in range(g * GROUP, (g + 1) * GROUP) if j < n_chunks]
                for g in range(n_groups)
            ]

            # pixels-per-bank: chunks sharing one PSUM bank in the arg tile
            ppb = 512 // F

            def emit_args(js, single_rowgroup=False):
                arg_ps = psum.tile([128, GROUP * F], f32, tag="arg", name="arg_ps")
                # adjacent arg matmuls run CONCURRENTLY on alternating PE row
                # groups; two concurrent matmuls writing the same PSUM bank
                # hang the device. Emit in bank-interleaved order so adjacent
                # matmuls always target different banks. The FIRST group uses
                # a single row group (p=0, serial — same-position matmuls
                # never co-issue) so the first exp gates only on the earliest
                # input DMA, not on the duplicate row-group copy.
                order = sorted(range(len(js)), key=lambda h: (h % ppb, h // ppb))
                for idx, h in enumerate(order):
                    j = js[h]
                    p = 0 if single_rowgroup else 32 * (idx % 2)
                    nc.tensor.matmul(
                        arg_ps[:, h * F : (h + 1) * F],
                        lhsT=cb_sb[p : p + K, F + j * 128 : F + (j + 1) * 128],
                        rhs=cb_sb[p : p + K, 0:F],
                        start=True,
                        stop=True,
                        tile_position=(p, 0),
                    )
                return arg_ps

            pend = None
            for rep in range(repeat):
                for gidx in range(n_groups):
                    js = group_js[gidx]
                    w = len(js) * F
                    arg_ps = (
                        pend
                        if pend is not None
                        else emit_args(js, single_rowgroup=(rep == 0 and gidx == 0))
                    )
                    pend = None
                    alpha_sb = work.tile([128, GROUP * F], f16, tag="alpha")
                    nc.scalar.activation(
                        alpha_sb[:, :w],
                        arg_ps[:, :w],
                        mybir.ActivationFunctionType.Exp,
                        bias=0.0,
                    )
                    if gidx + 1 < n_groups:
                        pend = emit_args(group_js[gidx + 1])
                    elif rep < repeat - 1:
                        pend = emit_args(group_js[0])
                    for h, j in enumerate(js):
                        o = ch_oct[j]
                        nc.tensor.matmul(
                            acc_region(o),
                            lhsT=aux_sb[:, M * j : M * (j + 1)],
                            rhs=alpha_sb[:, h * F : (h + 1) * F],
                            start=(first_of[o] == j and rep == 0),
                            stop=(last_of[o] == j and rep == repeat - 1),
                            tile_position=(0, 0),
                        )
                        if rep == repeat - 1:
                            for o2 in range(N_OCT):
                                if last_of[o2] != j:
                                    continue
                                nc.vector.tensor_copy(
                                    out=out_sb[:, o2 * F : (o2 + 1) * F],
                                    in_=acc_region(o2),
                                )
                                nc.sync.dma_start(
                                    out=out_ext[:, o2 * F : (o2 + 1) * F],
                                    in_=out_sb[:, o2 * F : (o2 + 1) * F],
                                )
    nc.compile()
    return nc


def _get_program(oct_chunks, repeat=1):
    key = (tuple(oct_chunks), repeat)
    if key not in _prog_cache:
        _prog_cache[key] = _build_program(list(oct_chunks), repeat)
    return _prog_cache[key]


def _coefs(means, stds, rhos, cxo, cyo):
    """[6, G] coefficients of -0.5*mahal2 in local coords; f64 intermediates."""
    sx = stds[:, 0].astype(np.float64)
    sy = stds[:, 1].astype(np.float64)
    r = rhos.astype(np.float64)
    om = 1.0 - r * r
    ia = 1.0 / (sx * sx * om)
    ib = -r / (sx * sy * om)
    ic = 1.0 / (sy * sy * om)
    mxl = means[:, 0].astype(np.float64) - cxo
    myl = means[:, 1].astype(np.float64) - cyo
    return np.stack(
        [
            -0.5 * ia,
            -ib,
            -0.5 * ic,
            ia * mxl + ib * myl,
            ib * mxl + ic * myl,
            -0.5 * (ia * mxl * mxl + 2 * ib * mxl * myl + ic * myl * myl),
        ],
        axis=0,
    )


def _basis():
    ys = np.arange(STRIP_ROWS, dtype=np.float64) + 0.5 - STRIP_ROWS / 2
    xs = np.arange(TILE_COLS, dtype=np.float64) + 0.5 - TILE_COLS / 2
    yl = np.repeat(ys, TILE_COLS)
    xl = np.tile(xs, STRIP_ROWS)
    return np.stack(
        [xl * xl, xl * yl, yl * yl, xl, yl, np.ones_like(xl)], axis=0
    ).astype(np.float16)  # exact in fp16 at these tile extents


def kernel(
    opacity,
    means,
    stds,
    rhos,
    colors,
    image_height,
    image_width,
    scale_factor,
    raster_ratio,
    _repeat=1,
):
    H = int(image_height)
    W = int(image_width)
    sf = float(scale_factor)
    rr = float(raster_ratio)
    opacity = np.asarray(opacity, np.float32)
    means = np.asarray(means, np.float32)
    stds = np.asarray(stds, np.float32) * np.float32(sf)
    rhos = np.asarray(rhos, np.float32)
    colors = np.asarray(colors, np.float32)

    n_tiles_y = H // STRIP_ROWS
    n_tiles_x = W // TILE_COLS
    n_tiles = n_tiles_y * n_tiles_x
    assert n_tiles % N_CORES == 0
    n_slots = n_tiles // N_CORES
    assert n_slots == N_SLOTS

    # --- host-side cull: EXACT rr-sigma ellipse vs tile pixel-center box.
    # The reference zeroes weights beyond rr sigma, so a gaussian whose
    # minimum Mahalanobis distance over the tile's pixel-center box exceeds
    # rr contributes exactly nothing — excluding it adds no error.
    mx = means[:, 0].astype(np.float64)
    my = means[:, 1].astype(np.float64)
    _sx = stds[:, 0].astype(np.float64)
    _sy = stds[:, 1].astype(np.float64)
    _r = rhos.astype(np.float64)
    _om = 1.0 - _r * _r
    _ia = 1.0 / (_sx * _sx * _om)
    _ic = 1.0 / (_sy * _sy * _om)
    _ib = -_r / (_sx * _sy * _om)

    def _min_mahal2(x0, x1, y0, y1):
        """min of ia*dx^2 + 2*ib*dx*dy + ic*dy^2 over the box (vectorized)."""
        dx0, dx1 = x0 - mx, x1 - mx
        dy0, dy1 = y0 - my, y1 - my
        inside = (dx0 <= 0) & (dx1 >= 0) & (dy0 <= 0) & (dy1 >= 0)
        best = np.full(len(mx), np.inf)
        for a in (dx0, dx1):
            dys = np.clip(-_ib * a / _ic, dy0, dy1)
            best = np.minimum(best, _ia * a * a + 2 * _ib * a * dys + _ic * dys * dys)
        for b in (dy0, dy1):
            dxs = np.clip(-_ib * b / _ia, dx0, dx1)
            best = np.minimum(best, _ia * dxs * dxs + 2 * _ib * dxs * b + _ic * b * b)
        return np.where(inside, 0.0, best)

    tile_ids = []  # per tile: gaussian index array
    tile_pos = []  # per tile: (ty, tx) pixel origin
    for tyi in range(n_tiles_y):
        ty = tyi * STRIP_ROWS
        for txi in range(n_tiles_x):
            tx = txi * TILE_COLS
            m2 = _min_mahal2(tx + 0.5, tx + TILE_COLS - 0.5, ty + 0.5, ty + STRIP_ROWS - 0.5)
            tile_ids.append(np.nonzero(m2 <= rr * rr)[0])
            tile_pos.append((ty, tx))

    # snake-deal tiles to cores by descending count, so every core gets a
    # near-identical sorted profile (SPMD: slot capacity is the max over
    # cores at each slot position)
    counts = [len(ids) for ids in tile_ids]
    t_order = sorted(range(n_tiles), key=lambda t: -counts[t])
    assign = [[] for _ in range(N_CORES)]
    for i, t in enumerate(t_order):
        rnd, pos = divmod(i, N_CORES)
        core = pos if rnd % 2 == 0 else N_CORES - 1 - pos
        assign[core].append(t)
    profile = [
        max(counts[assign[core][k]] for core in range(N_CORES)) for k in range(n_slots)
    ]

    octs = _octet_partition(profile)
    oct_sums = [sum(profile[i] for i in o) for o in octs]
    n_chunks, oct_chunks, oct_goff = _structure(oct_sums)
    tot = n_chunks * 128
    M = 3 * OCT

    nc = _get_program(oct_chunks, _repeat)

    basis6 = _basis()  # [6, F] fp16, exact
    lnop_all = np.where(
        opacity > 0, np.log(np.maximum(opacity.astype(np.float64), 1e-45)), -1e4
    )

    # global gaussian offset of each slot (canonical): octet o's slots are
    # packed consecutively from oct_goff[o] in octs[o] order
    slot_goff = {}
    slot_octpos = {}
    for o, slot_list in enumerate(octs):
        g = oct_goff[o]
        for s_pos, k in enumerate(slot_list):
            slot_goff[k] = g
            slot_octpos[k] = (o, s_pos)
            g += profile[k]

    in_maps = []
    for core in range(N_CORES):
        chi = np.zeros((6, tot), np.float64)
        clo = np.zeros((6, tot), np.float64)
        chi[5, :] = -1e4
        colvals = np.zeros((tot, 3), np.float16)
        aux_arr = np.zeros((128, M * n_chunks), np.float16)
        for k in range(n_slots):
            t = assign[core][k]
            ty, tx = tile_pos[t]
            ids = tile_ids[t]
            g = len(ids)
            if not g:
                continue
            base = slot_goff[k]
            cf = _coefs(
                means[ids], stds[ids], rhos[ids],
                tx + TILE_COLS / 2, ty + STRIP_ROWS / 2,
            )
            cf[5] += lnop_all[ids]  # fold ln(opacity): exp needs no bias
            c_hi16 = cf.astype(np.float16)
            chi[:, base : base + g] = c_hi16
            clo[:, base : base + g] = cf - c_hi16.astype(np.float64)
            colvals[base : base + g] = colors[ids]
            # scatter colors into the per-chunk lhsT columns
            o, s_pos = slot_octpos[k]
            for r in range(base, base + g):
                j = r // 128
                aux_arr[r - j * 128, M * j + 3 * s_pos : M * j + 3 * s_pos + 3] = (
                    colvals[r]
                )
        cb_arr = np.zeros((K, F + tot), np.float16)
        cb_arr[0:6, 0:F] = basis6
        cb_arr[6:12, 0:F] = basis6
        cb_arr[0:6, F:] = chi.astype(np.float16)
        cb_arr[6:12, F:] = clo.astype(np.float16)
        in_maps.append({"cb": cb_arr, "aux": aux_arr})

    global _last_in_maps
    _last_in_maps = in_maps
    run = _get_runner(nc)
    results = run(in_maps)

    out = np.zeros((H, W, 3), np.float32)
    for core in range(N_CORES):
        o_arr = results[core]["out"].astype(np.float32)  # [M, N_OCT*F]
        for k in range(n_slots):
            o, s_pos = slot_octpos[k]
            ty, tx = tile_pos[assign[core][k]]
            blk = o_arr[3 * s_pos : 3 * s_pos + 3, o * F : (o + 1) * F].reshape(
                3, STRIP_ROWS, TILE_COLS
            )
            out[ty : ty + STRIP_ROWS, tx : tx + TILE_COLS, :] = blk.transpose(1, 2, 0)
    if _repeat > 1:
        out /= np.float32(_repeat)
    return out
